# revision 9
# baseline (speedup 1.0000x reference)
"""3-layer GAT (BlastRadiusGNN) on 8 Trainium2 NeuronCores.

Sharding: dst-node octants (12500 nodes/core, padded to 12544 slots). Each
core owns the full softmax groups of its dst nodes (no cross-core stats);
node-feature tables are replicated via on-device AllGather between layers.

Edge processing uses an ELL-style layout: per core, dst slots are sorted by
in-degree (desc) so round r occupies exactly the slot prefix {s: deg[s]>r}
(prefix lengths maxed across cores so one SPMD program serves all 8).
Flat gather position j maps to (partition j%128, block j//128), which is
also the accumulator layout, so every segment op is an aligned vector op.
Self-loops are one extra round. Gathers use dma_gather (int16 idx) from
bf16 tables packing 4 nodes per row (idx = trow//4 < 25088); the quarter is
selected on-device with 2 shipped mask bits. Softmax does no max
subtraction (alphas are O(1)); pad slots are killed by a -1e9 additive mask
folded into the host-shipped alpha-edge stream.
"""
import hashlib
import numpy as np

N = 100000
E = 1600000
CORES = 8
OCT = 12500
SLOTS = 12544
NBLK_N = SLOTS // 128        # 98
PACK = 4
GROWS = CORES * SLOTS // PACK  # 25088 table rows (int16-safe)
NEG = 0.2
MASKV = -1e9
CHUNK_BLK = [64, 32, 64]
NODE_W = [32, 96, 32]        # bf16 node width per layer table
AS_OFF = [5, 64, 1]
SRC_W = [5, 32, 1]
SRC_SHARED = [True, False, True]
HEADS = [4, 2, 1]
ACC_W = [20, 64, 1]
ST_W = 9                      # alE1(4) alE2(2) alE3(1) b0 b1
ALE_OFF = [0, 4, 6]


def fold_params(params):
    (W1, aS1, aD1, We1, aE1, b1, W2, aS2, aD2, We2, aE2, b2,
     W3, aS3, aD3, We3, aE3, b3) = [np.asarray(p, np.float32) for p in params]

    def fold(Wm, a, H, C):
        return np.einsum("dhc,hc->dh", Wm.reshape(Wm.shape[0], H, C),
                         a).astype(np.float32)

    P = {
        "B": [fold(We1, aE1, 4, 32), fold(We2, aE2, 2, 32),
              fold(We3, aE3, 1, 1)],
        "As": [fold(W1, aS1, 4, 32), fold(W2, aS2, 2, 32),
               fold(W3, aS3, 1, 1)],
        "Ad": [fold(W1, aD1, 4, 32), fold(W2, aD2, 2, 32),
               fold(W3, aD3, 1, 1)],
        "W": [W1, W2, W3], "b": [b1, b2, b3],
    }
    W1s = np.zeros((20, 128), np.float32)
    for h in range(4):
        W1s[h * 5:(h + 1) * 5, h * 32:(h + 1) * 32] = W1[:, h * 32:(h + 1) * 32]
    P["W1s"] = W1s
    P["W2cat"] = np.concatenate([W2, P["As"][1], P["Ad"][1]], axis=1)  # 128x68
    P["W3cat"] = np.concatenate([W3, P["As"][2], P["Ad"][2]], axis=1)  # 64x3
    return P


def tpos_of_slot(s):
    """table position of slot s within an octant: p-major order."""
    return (s % 128) * NBLK_N + s // 128


def build_host(edge_index, edge_attr, x, P):
    src = np.asarray(edge_index[0], np.int64)
    dst = np.asarray(edge_index[1], np.int64)
    edge_attr = np.asarray(edge_attr, np.float32)
    x = np.asarray(x, np.float32)

    deg_in = np.bincount(dst, minlength=N).astype(np.float32)
    loop_attr = np.zeros((N, 2), np.float32)
    for k in range(2):
        loop_attr[:, k] = np.bincount(dst, weights=edge_attr[:, k],
                                      minlength=N)
    loop_attr /= np.maximum(deg_in, 1.0)[:, None]

    owner = dst // OCT
    slot_of = np.empty(N, np.int64)
    perms, deg_sls = [], []
    for c in range(CORES):
        m = owner == c
        deg = np.bincount(dst[m] - c * OCT, minlength=OCT)
        order = np.argsort(-deg, kind="stable")
        perm = np.full(SLOTS, -1, np.int64)
        perm[:OCT] = order
        perms.append(perm)
        s_of = np.empty(OCT, np.int64)
        s_of[order] = np.arange(OCT)
        slot_of[c * OCT:(c + 1) * OCT] = s_of
        dsl = np.zeros(SLOTS, np.int64)
        dsl[:OCT] = deg[order]
        deg_sls.append(dsl)

    # table row position: trow = owner*SLOTS + tpos(slot); p-major tpos
    all_slots = np.arange(SLOTS)
    tpos = tpos_of_slot(all_slots)
    trow = (np.arange(N) // OCT) * SLOTS + tpos[slot_of]

    # common round structure (max prefix across cores)
    maxdeg = int(max(d.max() for d in deg_sls))
    n_r = np.zeros(maxdeg, np.int64)
    for d in deg_sls:
        for r in range(maxdeg):
            n_r[r] = max(n_r[r], int((d > r).sum()))
    nblk_r = (n_r + 127) // 128
    blk_off = np.zeros(maxdeg + 1, np.int64)
    blk_off[1:] = np.cumsum(nblk_r)
    nblk_tot = int(blk_off[maxdeg]) + NBLK_N
    ns = nblk_tot * 128
    rounds = [(int(blk_off[r]), int(nblk_r[r])) for r in range(maxdeg)]
    rounds.append((int(blk_off[maxdeg]), NBLK_N))

    def chunkify(cb):
        chunks, cur = [], 0
        while cur < nblk_tot:
            nb = min(cb, nblk_tot - cur)
            runs = []
            for (b0, nblk) in rounds:
                lo, hi = max(cur, b0), min(cur + nb, b0 + nblk)
                if lo < hi:
                    runs.append((lo - cur, hi - lo, lo - b0))
            chunks.append((cur, nb, runs))
            cur += nb
        return chunks

    meta = {
        "nblk_tot": nblk_tot,
        "chunks": [chunkify(CHUNK_BLK[li]) for li in range(3)],
    }

    cores = []
    for c in range(CORES):
        m = owner == c
        e_src = src[m]
        e_slot = slot_of[dst[m]]
        e_ea = edge_attr[m]

        o = np.argsort(e_slot, kind="stable")
        so = e_slot[o]
        runstart = np.r_[0, np.nonzero(np.diff(so))[0] + 1]
        rid = np.zeros(len(so), np.int64)
        rid[runstart[1:]] = 1
        rid = np.cumsum(rid)
        rank = np.empty(len(so), np.int64)
        rank[o] = np.arange(len(so)) - runstart[rid]

        idx16 = np.zeros(ns, np.int16)
        st = np.zeros((ns, ST_W), np.float32)
        st[:, 0:7] = MASKV  # all alE cols masked by default
        st[:, 7:9] = 0.0

        flatpos = blk_off[rank] * 128 + e_slot
        tr = trow[e_src]
        idx16[flatpos] = (tr // PACK).astype(np.int16)
        q = tr % PACK
        for li in range(3):
            st[flatpos, ALE_OFF[li]:ALE_OFF[li] + HEADS[li]] = \
                e_ea @ P["B"][li]
        st[flatpos, 7] = q % 2
        st[flatpos, 8] = q // 2

        # self round
        sbase = int(blk_off[maxdeg]) * 128
        sl = all_slots
        tr_s = c * SLOTS + tpos
        idx16[sbase + sl] = (tr_s // PACK).astype(np.int16)
        la = np.zeros((SLOTS, 2), np.float32)
        real = perms[c] >= 0
        la[real] = loop_attr[c * OCT + perms[c][real]]
        for li in range(3):
            st[sbase + sl, ALE_OFF[li]:ALE_OFF[li] + HEADS[li]] = \
                la @ P["B"][li]
        st[sbase + sl, 7] = tr_s % 2
        st[sbase + sl, 8] = (tr_s % PACK) // 2

        # host-built L1 data (slot layouts)
        xs = np.zeros((SLOTS, 5), np.float32)
        xs[real] = x[c * OCT + perms[c][real]]
        aD1 = (xs @ P["Ad"][0]).astype(np.float32)
        node1 = np.zeros((SLOTS, NODE_W[0]), np.float32)
        node1[:, 0:5] = xs
        node1[:, 5:9] = xs @ P["As"][0]
        # t1 octant rows in tpos order: row r holds nodes tpos 4r..4r+3
        inv_t = np.empty(SLOTS, np.int64)
        inv_t[tpos] = all_slots          # tpos -> slot
        t1oct = node1[inv_t].reshape(SLOTS // PACK, PACK * NODE_W[0])

        def pmaj(a, w):
            return np.ascontiguousarray(
                a.reshape(nblk_tot, 128, w).transpose(1, 0, 2)).reshape(128, -1)

        import ml_dtypes
        bits = np.zeros((ns, 2), np.int8)
        bits[:, 0] = st[:, 7]
        bits[:, 1] = st[:, 8]
        cores.append({
            "idx16": np.ascontiguousarray(idx16.reshape(-1, 16).T),
            "bits": pmaj(bits, 2).astype(np.int8),
            "st": pmaj(st, ST_W).astype(ml_dtypes.bfloat16),
            "aD1": np.ascontiguousarray(
                aD1.reshape(NBLK_N, 128, 4).transpose(1, 0, 2)).reshape(128, -1),
            "t1oct": t1oct,
            "perm": perms[c],
        })
    return cores, meta


# -------------------- device program --------------------

def _split_waits(nc):
    import concourse.mybir as mybir
    ctr = [0]
    for bb in nc.main_func.blocks:
        il = bb.instructions
        out, changed = [], False
        for inst in il:
            si = inst.sync_info
            if si is not None and len(si.on_wait) > 1:
                waits = list(si.on_wait)
                for w in waits[:-1]:
                    ctr[0] += 1
                    nop = mybir.InstNoOp(name=f"W-split-{ctr[0]}", ins=[],
                                         outs=[])
                    nop.engine = inst.engine
                    nop.sync_info = mybir.SyncInfo(on_wait=[w], on_update=[])
                    out.append(nop)
                inst.sync_info = mybir.SyncInfo(
                    on_wait=[waits[-1]], on_update=list(si.on_update))
                changed = True
            out.append(inst)
        if changed:
            bb.instructions = out


def build_program(meta, debug=False):
    import concourse.bacc as bacc
    import concourse.mybir as mybir
    import concourse.tile as tile
    from concourse.bass import AP
    from concourse.masks import make_identity

    f32 = mybir.dt.float32
    bf16 = mybir.dt.bfloat16
    i16 = mybir.dt.int16
    AL = mybir.AluOpType
    AF = mybir.ActivationFunctionType
    nbt = meta["nblk_tot"]
    ns = nbt * 128

    nc = bacc.Bacc("TRN2", target_bir_lowering=False, debug=False,
                   num_devices=CORES)

    d_idx = nc.dram_tensor("idx16", [16, ns // 16], i16, kind="ExternalInput")
    d_st = nc.dram_tensor("st", [128, nbt * ST_W], bf16, kind="ExternalInput")
    i8 = mybir.dt.int8
    d_bits = nc.dram_tensor("bits", [128, nbt * 2], i8, kind="ExternalInput")
    d_aD1 = nc.dram_tensor("aD1", [128, NBLK_N * 4], f32,
                           kind="ExternalInput")
    d_t1o = nc.dram_tensor("t1oct", [SLOTS // PACK, PACK * NODE_W[0]], f32,
                           kind="ExternalInput")
    d_w1s = nc.dram_tensor("W1s", [20, 128], f32, kind="ExternalInput")
    d_w2c = nc.dram_tensor("W2cat", [128, 68], f32, kind="ExternalInput")
    d_w3c = nc.dram_tensor("W3cat", [64, 3], f32, kind="ExternalInput")
    d_b1 = nc.dram_tensor("b1c", [128, 1], f32, kind="ExternalInput")
    d_b2 = nc.dram_tensor("b2r", [128, 64], f32, kind="ExternalInput")
    d_b3 = nc.dram_tensor("b3s", [128, 1], f32, kind="ExternalInput")
    d_probs = nc.dram_tensor("probs", [128, NBLK_N], f32,
                             kind="ExternalOutput")
    if debug:
        d_dbg = [nc.dram_tensor(f"dbg{li}", [128, NBLK_N * (ACC_W[li] +
                                                            HEADS[li])],
                                f32, kind="ExternalOutput")
                 for li in range(3)]

    OROWS = SLOTS // PACK  # 3136
    t_in = [nc.dram_tensor(f"tg_in{li}", [OROWS, PACK * NODE_W[li]], bf16)
            for li in range(3)]
    t_full = [nc.dram_tensor(f"tg_full{li}", [GROWS, PACK * NODE_W[li]],
                             bf16, addr_space="Shared") for li in range(3)]

    with tile.TileContext(nc) as tc:
        with tc.tile_pool(name="perm", bufs=1) as ppool, \
             tc.tile_pool(name="acc", bufs=1) as apool, \
             tc.tile_pool(name="gat", bufs=2) as gpool, \
             tc.tile_pool(name="wrk", bufs=2) as wpool, \
             tc.tile_pool(name="eps", bufs=2) as epool, \
             tc.tile_pool(name="ps", bufs=2, space="PSUM") as pspool:

            t_idx = ppool.tile([128, ns // 16], i16)
            for g in range(8):
                nc.sync.dma_start(out=t_idx[16 * g:16 * (g + 1), :],
                                  in_=d_idx[:, :])
            ident = ppool.tile([128, 128], f32)
            make_identity(nc, ident[:])
            t_w1f = ppool.tile([20, 128], f32)
            nc.sync.dma_start(out=t_w1f[:], in_=d_w1s[:, :])
            t_w1s = ppool.tile([20, 128], bf16)
            nc.vector.tensor_copy(out=t_w1s[:], in_=t_w1f[:])
            t_w2f = ppool.tile([128, 68], f32)
            nc.sync.dma_start(out=t_w2f[:], in_=d_w2c[:, :])
            t_w2c = ppool.tile([128, 68], bf16)
            nc.vector.tensor_copy(out=t_w2c[:], in_=t_w2f[:])
            t_w3f = ppool.tile([64, 3], f32)
            nc.sync.dma_start(out=t_w3f[:], in_=d_w3c[:, :])
            t_w3c = ppool.tile([64, 3], bf16)
            nc.vector.tensor_copy(out=t_w3c[:], in_=t_w3f[:])
            t_b1 = ppool.tile([128, 1], f32)
            nc.sync.dma_start(out=t_b1[:], in_=d_b1[:, :])
            t_b2 = ppool.tile([128, 64], f32)
            nc.sync.dma_start(out=t_b2[:], in_=d_b2[:, :])
            t_b3 = ppool.tile([128, 1], f32)
            nc.sync.dma_start(out=t_b3[:], in_=d_b3[:, :])
            t_aD = ppool.tile([128, NBLK_N * 4], f32, tag="aD")
            nc.sync.dma_start(out=t_aD[:, :], in_=d_aD1[:, :])

            # t1 octant -> bf16 -> bounce -> AllGather
            ow = OROWS * PACK * NODE_W[0] // 128
            s_oct = gpool.tile([128, ow], f32, tag="gC")
            nc.sync.dma_start(
                out=s_oct[:],
                in_=d_t1o[:, :].flatten().rearrange("(p f) -> p f", p=128))
            s_octb = gpool.tile([128, ow], bf16, tag="gC")
            nc.vector.tensor_copy(out=s_octb[:], in_=s_oct[:])
            nc.sync.dma_start(
                out=t_in[0][:, :].flatten().rearrange("(p f) -> p f", p=128),
                in_=s_octb[:])
            nc.gpsimd.collective_compute(
                "AllGather", AL.bypass,
                replica_groups=[list(range(CORES))],
                ins=[t_in[0][:].opt()], outs=[t_full[0][:].opt()])

            def bc(ap, k):
                """broadcast a trailing unit free dim to k via 0-stride"""
                a = ap.copy()
                lst = list(a.ap)
                assert lst[-1][1] == 1
                lst[-1] = (0, k)
                return AP(a.tensor, a.offset, lst)

            def bcmid(ap2d, nb):
                """[P, W] -> [P, nb, W] via 0-stride middle dim"""
                a = ap2d.copy()
                lst = list(a.ap)
                lst = [lst[0], (0, nb), lst[1]]
                return AP(a.tensor, a.offset, lst)

            for li in range(3):
                H, sw, aw = HEADS[li], SRC_W[li], ACC_W[li]
                nw = NODE_W[li]
                row_w = PACK * nw
                used = AS_OFF[li] + H
                t_acc = apool.tile([128, NBLK_N * max(ACC_W)], f32,
                                   tag="accA")
                t_den = apool.tile([128, NBLK_N * max(HEADS)], f32,
                                   tag="denA")
                nc.vector.memset(t_acc[:, :NBLK_N * aw], 0.0)
                nc.vector.memset(t_den[:, :NBLK_N * H], 0.0)
                acc3 = t_acc[:, :NBLK_N * aw].rearrange(
                    "p (b w) -> p b w", w=aw)
                den3 = t_den[:, :NBLK_N * H].rearrange(
                    "p (b h) -> p b h", h=H)
                aD3 = t_aD[:, :NBLK_N * H].rearrange("p (b h) -> p b h", h=H)

                for (cb0, cnb, runs) in meta["chunks"][li]:
                    t_g = gpool.tile([128, max(CHUNK_BLK[0] * PACK * NODE_W[0], CHUNK_BLK[1] * PACK * NODE_W[1], CHUNK_BLK[2] * PACK * NODE_W[2])], bf16,
                                     tag="gC")
                    nc.gpsimd.dma_gather(
                        out_ap=t_g[:, :cnb * row_w].rearrange(
                            "p (b e) -> p b e", e=row_w),
                        in_ap=t_full[li][:],
                        idxs_ap=t_idx[:16, cb0 * 8:(cb0 + cnb) * 8],
                        num_idxs=cnb * 128,
                        num_idxs_reg=cnb * 128,
                        elem_size=row_w,
                        single_packet=False,
                    )
                    g4 = t_g[:, :cnb * row_w].rearrange(
                        "p (b q e) -> p b q e", q=PACK, e=nw)
                    t_st = wpool.tile([128, CHUNK_BLK[li] * ST_W], bf16,
                                      tag="stC")
                    nc.sync.dma_start(
                        out=t_st[:, :cnb * ST_W],
                        in_=d_st[:, cb0 * ST_W:(cb0 + cnb) * ST_W])
                    st3 = t_st[:, :cnb * ST_W].rearrange(
                        "p (b w) -> p b w", w=ST_W)
                    t_bit = wpool.tile([128, max(CHUNK_BLK) * 2], i8,
                                       tag="bitC")
                    nc.sync.dma_start(
                        out=t_bit[:, :cnb * 2],
                        in_=d_bits[:, cb0 * 2:(cb0 + cnb) * 2])
                    bit3 = t_bit[:, :cnb * 2].rearrange(
                        "p (b w) -> p b w", w=2)

                    t_sel = wpool.tile([128, max(CHUNK_BLK[i] * (AS_OFF[i] + HEADS[i]) for i in range(3))], bf16,
                                       tag="selC")
                    sel3 = t_sel[:, :cnb * used].rearrange(
                        "p (b w) -> p b w", w=used)
                    t_h1 = wpool.tile([128, max(CHUNK_BLK[i] * (AS_OFF[i] + HEADS[i]) for i in range(3))], bf16,
                                      tag="h1C")
                    h13 = t_h1[:, :cnb * used].rearrange(
                        "p (b w) -> p b w", w=used)
                    mb0 = bc(bit3[:, :, 0:1], used)
                    mb1 = bc(bit3[:, :, 1:2], used)
                    nc.vector.select(sel3, mb0, g4[:, :, 1, :used],
                                     g4[:, :, 0, :used])
                    nc.vector.select(h13, mb0, g4[:, :, 3, :used],
                                     g4[:, :, 2, :used])
                    nc.vector.copy_predicated(sel3, mb1, h13)

                    t_ex = wpool.tile([128, max(CHUNK_BLK[0] * HEADS[0], CHUNK_BLK[1] * HEADS[1], CHUNK_BLK[2] * HEADS[2])], f32,
                                      tag="exC")
                    ex3 = t_ex[:, :cnb * H].rearrange(
                        "p (b h) -> p b h", h=H)
                    nc.vector.tensor_tensor(
                        out=ex3, in0=sel3[:, :, AS_OFF[li]:AS_OFF[li] + H],
                        in1=st3[:, :, ALE_OFF[li]:ALE_OFF[li] + H],
                        op=AL.add)
                    for (rb0, rnb, ab0) in runs:
                        nc.vector.tensor_tensor(
                            out=ex3[:, rb0:rb0 + rnb, :],
                            in0=ex3[:, rb0:rb0 + rnb, :],
                            in1=aD3[:, ab0:ab0 + rnb, :], op=AL.add)
                    nc.scalar.activation(out=t_ex[:, :cnb * H],
                                         in_=t_ex[:, :cnb * H],
                                         func=AF.Lrelu, alpha=NEG)
                    nc.scalar.activation(out=t_ex[:, :cnb * H],
                                         in_=t_ex[:, :cnb * H], func=AF.Exp)
                    for (rb0, rnb, ab0) in runs:
                        nc.vector.tensor_tensor(
                            out=den3[:, ab0:ab0 + rnb, :],
                            in0=den3[:, ab0:ab0 + rnb, :],
                            in1=ex3[:, rb0:rb0 + rnb, :], op=AL.add)
                    t_sc = wpool.tile([128, max(CHUNK_BLK[0] * ACC_W[0], CHUNK_BLK[1] * ACC_W[1], CHUNK_BLK[2] * ACC_W[2])], bf16,
                                      tag="scC")
                    sc3 = t_sc[:, :cnb * aw].rearrange(
                        "p (b w) -> p b w", w=aw)
                    for h in range(H):
                        soff = 0 if SRC_SHARED[li] else h * sw
                        nc.vector.tensor_tensor(
                            out=sc3[:, :, h * sw:(h + 1) * sw],
                            in0=sel3[:, :, soff:soff + sw],
                            in1=bc(ex3[:, :, h:h + 1], sw), op=AL.mult)
                    for (rb0, rnb, ab0) in runs:
                        nc.vector.tensor_tensor(
                            out=acc3[:, ab0:ab0 + rnb, :],
                            in0=acc3[:, ab0:ab0 + rnb, :],
                            in1=sc3[:, rb0:rb0 + rnb, :], op=AL.add)

                if debug:
                    nc.sync.dma_start(
                        out=d_dbg[li][:, :NBLK_N * aw],
                        in_=t_acc[:, :NBLK_N * aw])
                    nc.sync.dma_start(
                        out=d_dbg[li][:, NBLK_N * aw:NBLK_N * (aw + H)],
                        in_=t_den[:, :NBLK_N * H])

                # ---- normalize ----
                t_rec = apool.tile([128, NBLK_N * max(HEADS)], f32,
                                   tag="recA")
                nc.vector.reciprocal(t_rec[:, :NBLK_N * H],
                                     t_den[:, :NBLK_N * H])
                rec3 = t_rec[:, :NBLK_N * H].rearrange(
                    "p (b h) -> p b h", h=H)
                for h in range(H):
                    nc.vector.tensor_tensor(
                        out=acc3[:, :, h * sw:(h + 1) * sw],
                        in0=acc3[:, :, h * sw:(h + 1) * sw],
                        in1=bc(rec3[:, :, h:h + 1], sw), op=AL.mult)

                # ---- epilogue ----
                if li == 0:
                    # out1T = W1s.T @ accnT per 512-row group; +b1; ELU
                    t_xT = apool.tile([128, SLOTS], bf16, tag="xT")
                    for g0 in range(0, NBLK_N, 4):
                        gn = min(4, NBLK_N - g0)
                        ps_t = pspool.tile([32, 512], f32, tag="pA")
                        for k in range(gn):
                            nc.tensor.transpose(
                                out=ps_t[:20, k * 128:(k + 1) * 128],
                                in_=acc3[:, g0 + k, :], identity=ident[:])
                        s_aT = epool.tile([32, 512], bf16, tag="aT")
                        nc.vector.tensor_copy(out=s_aT[:20, :gn * 128],
                                              in_=ps_t[:20, :gn * 128])
                        ps_o = pspool.tile([128, 512], f32, tag="pB")
                        nc.tensor.matmul(
                            out=ps_o[:, :gn * 128], lhsT=t_w1s[:],
                            rhs=s_aT[:20, :gn * 128], start=True, stop=True)
                        nc.vector.tensor_tensor(
                            out=t_xT[:, g0 * 128:(g0 + gn) * 128],
                            in0=ps_o[:, :gn * 128],
                            in1=bc(t_b1[:], gn * 128), op=AL.add)
                    # ELU in place on feature-major xin2 (chunked tmp)
                    t_tmp = apool.tile([128, SLOTS // 4], f32, tag="tmpX")
                    for q0 in range(0, SLOTS, SLOTS // 4):
                        qs = slice(q0, q0 + SLOTS // 4)
                        nc.vector.tensor_scalar_min(
                            out=t_tmp[:], in0=t_xT[:, qs], scalar1=0.0)
                        nc.scalar.activation(out=t_tmp[:], in_=t_tmp[:],
                                             func=AF.Exp)
                        nc.vector.tensor_scalar_add(
                            out=t_tmp[:], in0=t_tmp[:], scalar1=-1.0)
                        nc.vector.tensor_tensor(
                            out=t_xT[:, qs], in0=t_xT[:, qs], in1=t_tmp[:],
                            op=AL.max)
                    # table2 rows: per 128-node block, lhsT = xT slice
                    t_stage = apool.tile([128, NBLK_N * NODE_W[1]], bf16,
                                         tag="stgA")
                    nc.vector.memset(t_stage[:], 0.0)
                    stg3 = t_stage[:].rearrange("p (b f) -> p b f",
                                                f=NODE_W[1])
                    for b in range(NBLK_N):
                        ps_o = pspool.tile([128, 68], f32, tag="pC")
                        nc.tensor.matmul(
                            out=ps_o[:], lhsT=t_xT[:, b * 128:(b + 1) * 128],
                            rhs=t_w2c[:], start=True, stop=True)
                        nc.vector.tensor_copy(out=stg3[:, b, 0:66],
                                              in_=ps_o[:, 0:66])
                        nc.vector.tensor_copy(out=t_aD[:, b * 2:b * 2 + 2],
                                              in_=ps_o[:, 66:68])
                    nc.sync.dma_start(
                        out=t_in[1][:, :].flatten().rearrange(
                            "(p f) -> p f", p=128),
                        in_=t_stage[:])
                    nc.gpsimd.collective_compute(
                        "AllGather", AL.bypass,
                        replica_groups=[list(range(CORES))],
                        ins=[t_in[1][:].opt()], outs=[t_full[1][:].opt()])
                elif li == 1:
                    # out2 = accn + b2; ELU; table3 via per-block transpose
                    for b in range(0, NBLK_N, 7):
                        nb = min(7, NBLK_N - b)
                        nc.vector.tensor_tensor(
                            out=acc3[:, b:b + nb, :],
                            in0=acc3[:, b:b + nb, :],
                            in1=bcmid(t_b2[:], nb), op=AL.add)
                    t_tmp = apool.tile([128, SLOTS // 4], f32, tag="tmpX")
                    qw = NBLK_N * 64 // 2
                    for q0 in range(0, NBLK_N * 64, qw):
                        qs = slice(q0, q0 + qw)
                        nc.vector.tensor_scalar_min(
                            out=t_tmp[:, :qw], in0=t_acc[:, qs], scalar1=0.0)
                        nc.scalar.activation(out=t_tmp[:, :qw],
                                             in_=t_tmp[:, :qw], func=AF.Exp)
                        nc.vector.tensor_scalar_add(
                            out=t_tmp[:, :qw], in0=t_tmp[:, :qw], scalar1=-1.0)
                        nc.vector.tensor_tensor(
                            out=t_acc[:, qs], in0=t_acc[:, qs],
                            in1=t_tmp[:, :qw], op=AL.max)
                    t_stage = apool.tile([128, NBLK_N * NODE_W[2]], bf16,
                                         tag="stgA")
                    nc.vector.memset(t_stage[:], 0.0)
                    stg3 = t_stage[:].rearrange("p (b f) -> p b f",
                                                f=NODE_W[2])
                    for b in range(NBLK_N):
                        ps_t = pspool.tile([64, 128], f32, tag="pA")
                        nc.tensor.transpose(
                            out=ps_t[:64, :], in_=acc3[:, b, :],
                            identity=ident[:])
                        s_l = epool.tile([64, 128], bf16, tag="l3")
                        nc.vector.tensor_copy(out=s_l[:], in_=ps_t[:64, :])
                        ps_o = pspool.tile([128, 3], f32, tag="pC")
                        nc.tensor.matmul(out=ps_o[:], lhsT=s_l[:],
                                         rhs=t_w3c[:], start=True, stop=True)
                        nc.vector.tensor_copy(out=stg3[:, b, 0:2],
                                              in_=ps_o[:, 0:2])
                        nc.vector.tensor_copy(out=t_aD[:, b:b + 1],
                                              in_=ps_o[:, 2:3])
                    nc.sync.dma_start(
                        out=t_in[2][:, :].flatten().rearrange(
                            "(p f) -> p f", p=128),
                        in_=t_stage[:])
                    nc.gpsimd.collective_compute(
                        "AllGather", AL.bypass,
                        replica_groups=[list(range(CORES))],
                        ins=[t_in[2][:].opt()], outs=[t_full[2][:].opt()])
                else:
                    t_pr = apool.tile([128, NBLK_N], f32, tag="prA")
                    nc.scalar.activation(
                        out=t_pr[:], in_=t_acc[:, :NBLK_N],
                        func=AF.Sigmoid, bias=t_b3[:])
                    nc.sync.dma_start(out=d_probs[:, :], in_=t_pr[:])

    nc.compile()
    _split_waits(nc)
    return nc


# -------------------- runner --------------------

_CACHE = {}


def _graph_key(edge_index):
    ei = np.asarray(edge_index)
    h = hashlib.sha1()
    h.update(str(ei.shape).encode())
    h.update(ei[:, :1024].tobytes())
    return h.hexdigest()


def _prep_in_maps(cores, P):
    b1 = P["b"][0].reshape(128, 1).astype(np.float32)
    b2 = np.tile(P["b"][1].reshape(1, 64), (128, 1)).astype(np.float32)
    b3 = np.tile(P["b"][2].reshape(1, 1), (128, 1)).astype(np.float32)
    return [{
        "idx16": c["idx16"], "st": c["st"], "aD1": c["aD1"],
        "bits": c["bits"],
        "t1oct": c["t1oct"].astype(np.float32),
        "W1s": P["W1s"], "W2cat": P["W2cat"], "W3cat": P["W3cat"],
        "b1c": b1, "b2r": b2, "b3s": b3,
    } for c in cores]


def kernel(x, edge_index, edge_attr,
           W1, aS1, aD1, We1, aE1, b1,
           W2, aS2, aD2, We2, aE2, b2,
           W3, aS3, aD3, We3, aE3, b3):
    from concourse.bass_utils import run_bass_kernel_spmd

    x = np.asarray(x, np.float32)
    edge_attr = np.asarray(edge_attr, np.float32)
    params = (W1, aS1, aD1, We1, aE1, b1, W2, aS2, aD2, We2, aE2, b2,
              W3, aS3, aD3, We3, aE3, b3)
    P = fold_params(params)
    cores, meta = build_host(edge_index, edge_attr, x, P)

    key = (_graph_key(edge_index), meta["nblk_tot"])
    if key not in _CACHE:
        _CACHE[key] = build_program(meta)
    nc = _CACHE[key]

    in_maps = _prep_in_maps(cores, P)
    res = run_bass_kernel_spmd(nc, in_maps, list(range(CORES)))

    out = np.zeros(N, np.float32)
    sl = np.arange(SLOTS)
    for c in range(CORES):
        pr = np.asarray(res.results[c]["probs"])  # [128, 98]
        probs_sl = pr[sl % 128, sl // 128]
        perm = cores[c]["perm"]
        real = perm >= 0
        out[c * OCT + perm[real]] = probs_sl[real]
    return out
